# revision 9
# baseline (speedup 1.0000x reference)
"""Causal self-attention (B=4, T=2048, C=2048, nH=16) on 8 trn2 NeuronCores.

Sharding: core r -> (batch b = r//2, head-group g = r%2, 8 heads per group).
Each core computes qkv projection for its heads, causal attention, and a
partial out-projection over its local 1024 c_in columns. The host sums the
two partials per batch and adds bout (pure gather work, no collective).

All matmuls run as float32r (TF32) on the tensor engine; values are kept in
fp32 everywhere else.
"""

import sys

sys.path.insert(0, "/opt/trn_rl_repo")

import numpy as np

import concourse.bass as bass  # noqa: F401  (registers engine methods)
import concourse.mybir as mybir
import concourse.tile as tile
from concourse import bacc
from concourse.bass_utils import run_bass_kernel_spmd
from concourse.masks import make_identity

F32 = mybir.dt.float32
F32R = mybir.dt.float32r
P = 128


def build_nc(T=2048, C=2048, HPC=8, HD=128, debug=False):
    """One SPMD NeuronCore program. HPC = heads per core."""
    TQ = 512                 # q-tile / free-dim width
    KS = C // P              # contraction subtiles over C
    NT = T // TQ             # 512-wide t tiles
    NTB = T // P             # 128-wide t blocks
    CL = HPC * HD            # local section width (q, k or v cols)
    NQK = 2 * CL // P        # c-blocks for q+k projection
    WV = min(TQ, CL)         # v projection free width
    NCV = CL // WV
    KSL = CL // P            # out-proj contraction subtiles
    NCO = C // TQ            # out-proj c_out tiles
    DJ = TQ // P             # k-blocks per diagonal q-tile group
    scale = float(1.0 / np.sqrt(HD))

    nc = bacc.Bacc(None, target_bir_lowering=False, debug=debug)

    xT = nc.dram_tensor("xT", [C, T], F32, kind="ExternalInput")
    wqkT = nc.dram_tensor("wqkT", [C, 2 * CL], F32, kind="ExternalInput")
    wvT = nc.dram_tensor("wvT", [C, CL], F32, kind="ExternalInput")
    bqk = nc.dram_tensor("bqk", [2 * CL], F32, kind="ExternalInput")
    bv = nc.dram_tensor("bv", [1, CL], F32, kind="ExternalInput")
    woutT = nc.dram_tensor("woutT", [CL, C], F32, kind="ExternalInput")
    yp = nc.dram_tensor("yp", [T, C], F32, kind="ExternalOutput")
    k_out = nc.dram_tensor("k_out", [HPC, T, HD], F32, kind="ExternalOutput")
    v_out = nc.dram_tensor("v_out", [HPC, T, HD], F32, kind="ExternalOutput")

    x3 = xT.rearrange("(s p) t -> p s t", p=P)         # [P, KS, T]
    wqk3 = wqkT.rearrange("(s p) c -> p s c", p=P)     # [P, KS, 2CL]
    wv3 = wvT.rearrange("(s p) c -> p s c", p=P)       # [P, KS, CL]
    bqk2 = bqk.rearrange("(b p) -> p b", p=P)          # [P, NQK]
    wout3 = woutT.rearrange("(s p) c -> p s c", p=P)   # [P, KSL, C]

    with tile.TileContext(nc) as tc:
        with (
            tc.tile_pool(name="const", bufs=1) as const,
            tc.tile_pool(name="dram", bufs=1, space="DRAM") as dram,
        ):
            st_col = const.tile([P, 1], F32)
            nc.gpsimd.memset(st_col[:], 1.0)
            ones_col = const.tile([P, 1], F32R)        # column-sum lhsT
            nc.vector.tensor_copy(ones_col[:], st_col[:])
            ident = const.tile([P, P], F32)
            make_identity(nc, ident)
            # causal masks: mask[r][a, b] = 1.0 if (a + P*r <= b) else 0.0
            masks = []
            for r in range(DJ):
                m = const.tile([P, TQ], F32, tag=f"mask{r}")
                nc.gpsimd.memset(m[:], 0.0)
                nc.gpsimd.affine_select(
                    out=m[:], in_=m[:],
                    compare_op=mybir.AluOpType.is_gt,
                    fill=1.0,
                    base=P * r,
                    pattern=[[-1, TQ]],
                    channel_multiplier=1,
                )  # iota = P*r + a - b ; iota > 0 ? in_(0) : fill(1)
                masks.append(m)
            bqk_sb = const.tile([P, NQK], F32)
            nc.sync.dma_start(bqk_sb[:], bqk2[:])
            bv_sb = const.tile([1, CL], F32)
            nc.sync.dma_start(bv_sb[:], bv[:])
            bvb = const.tile([P, CL], F32)             # bias broadcast to 128 rows
            nc.gpsimd.partition_broadcast(bvb[:], bv_sb[:])

            qkT_d = dram.tile([NQK, P, T], F32)        # spilled qT/kT blocks
            attT_d = dram.tile([KSL, P, T], F32)       # attention out, [c_local, t]

            with tc.tile_pool(name="xpool", bufs=1) as xpool:
                x_sb = xpool.tile([P, KS, T], F32R)
                nc.sync.dma_start(x_sb[:], x3[:].bitcast(F32R))

                # ---------- Phase P1: v projection (normal orientation) ----------
                with (
                    tc.tile_pool(name="p1w", bufs=1) as p1w,
                    tc.tile_pool(name="p1s", bufs=3) as p1s,
                    tc.tile_pool(name="p1ps", bufs=4, space="PSUM") as p1ps,
                ):
                    for cv in range(NCV):
                        wv_sb = p1w.tile([P, KS, WV], F32R, tag="wv")
                        nc.sync.dma_start(
                            wv_sb[:], wv3[:, :, cv * WV:(cv + 1) * WV].bitcast(F32R))
                        for tb in range(NTB):
                            ps = p1ps.tile([P, WV], F32, tag="v")
                            for s in range(KS):
                                nc.tensor.matmul(
                                    ps[:], x_sb[:, s, tb * P:(tb + 1) * P],
                                    wv_sb[:, s, :],
                                    start=(s == 0), stop=(s == KS - 1))
                            ev = p1s.tile([P, WV], F32, tag="vev")
                            nc.vector.tensor_add(
                                ev[:], ps[:], bvb[:, cv * WV:(cv + 1) * WV])
                            nc.sync.dma_start(
                                v_out[cv * WV // HD:(cv + 1) * WV // HD,
                                      tb * P:(tb + 1) * P, :]
                                .rearrange("h p d -> p h d"),
                                ev[:].rearrange("p (h d) -> p h d", d=HD))

                # ---------- Phase P2: qT / kT projection (transposed) ----------
                with (
                    tc.tile_pool(name="p2w", bufs=2) as p2w,
                    tc.tile_pool(name="p2s", bufs=3) as p2s,
                    tc.tile_pool(name="p2ps", bufs=4, space="PSUM") as p2ps,
                    tc.tile_pool(name="p2tps", bufs=2, space="PSUM") as p2tps,
                ):
                    for cb in range(NQK):
                        w_sb = p2w.tile([P, KS, P], F32R, tag="wqk")
                        nc.sync.dma_start(
                            w_sb[:], wqk3[:, :, cb * P:(cb + 1) * P].bitcast(F32R))
                        for tt in range(NT):
                            ps = p2ps.tile([P, TQ], F32, tag="qk")
                            for s in range(KS):
                                nc.tensor.matmul(
                                    ps[:], w_sb[:, s, :],
                                    x_sb[:, s, tt * TQ:(tt + 1) * TQ],
                                    start=(s == 0), stop=(s == KS - 1))
                            ev = p2s.tile([P, TQ], F32, tag="qkev")
                            nc.scalar.activation(
                                ev[:], ps[:], mybir.ActivationFunctionType.Identity,
                                bias=bqk_sb[:, cb:cb + 1])
                            nc.sync.dma_start(
                                qkT_d[cb, :, tt * TQ:(tt + 1) * TQ], ev[:])
                            if cb >= NQK // 2:  # k block: also emit normal-layout k
                                h = cb - NQK // 2
                                for u in range(TQ // P):
                                    tps = p2tps.tile([P, P], F32, tag="tr")
                                    nc.tensor.transpose(
                                        tps[:], ev[:, u * P:(u + 1) * P], ident[:])
                                    tev = p2s.tile([P, P], F32, tag="trev")
                                    nc.vector.tensor_copy(tev[:], tps[:])
                                    t0 = tt * TQ + u * P
                                    nc.sync.dma_start(
                                        k_out[h, t0:t0 + P, :], tev[:])

            # ---------- Phase A: causal attention per head ----------
            with (
                tc.tile_pool(name="aq", bufs=2) as aq,
                tc.tile_pool(name="as", bufs=4) as asb,
                tc.tile_pool(name="aps_s", bufs=3, space="PSUM") as aps_s,
                tc.tile_pool(name="aps_y", bufs=2, space="PSUM") as aps_y,
                tc.tile_pool(name="aps_d", bufs=1, space="PSUM") as aps_d,
            ):
                for h in range(HPC):
                    q_sb = aq.tile([P, T], F32R, tag="q")
                    nc.sync.dma_start(q_sb[:], qkT_d[h].bitcast(F32R))
                    k_sb = aq.tile([P, T], F32R, tag="k")
                    nc.sync.dma_start(k_sb[:], qkT_d[HPC + h].bitcast(F32R))
                    v_sb = aq.tile([P, NTB, HD], F32R, tag="v")
                    nc.sync.dma_start(
                        v_sb[:],
                        v_out[h].rearrange("(tb p) d -> p tb d", p=P).bitcast(F32R))
                    for qt in range(NT):
                        jmax = (qt + 1) * DJ
                        ps_y = aps_y.tile([HD, TQ], F32, tag="y")
                        ps_sum = aps_d.tile([1, TQ], F32, tag="sum")
                        for j in range(jmax):
                            ps_S = aps_s.tile([P, TQ], F32, tag="S")
                            nc.tensor.matmul(
                                ps_S[:], k_sb[:, j * P:(j + 1) * P],
                                q_sb[:, qt * TQ:(qt + 1) * TQ],
                                start=True, stop=True)
                            p_t = asb.tile([P, TQ], F32R, tag="p")
                            nc.scalar.activation(
                                p_t[:], ps_S[:],
                                mybir.ActivationFunctionType.Exp, scale=scale)
                            r = j - qt * DJ
                            if r >= 0:
                                nc.vector.tensor_mul(p_t[:], p_t[:], masks[r][:])
                            nc.tensor.matmul(
                                ps_y[:], v_sb[:, j, :], p_t[:],
                                start=(j == 0), stop=(j == jmax - 1))
                            nc.tensor.matmul(
                                ps_sum[:], ones_col[:], p_t[:],
                                start=(j == 0), stop=(j == jmax - 1))
                        recip = asb.tile([1, TQ], F32, tag="recip")
                        nc.vector.reciprocal(recip[:], ps_sum[:])
                        binv = asb.tile([P, TQ], F32, tag="binv")
                        nc.gpsimd.partition_broadcast(binv[:], recip[:])
                        att_t = asb.tile([HD, TQ], F32, tag="att")
                        nc.vector.tensor_mul(att_t[:], ps_y[:], binv[:])
                        nc.sync.dma_start(
                            attT_d[h, :, qt * TQ:(qt + 1) * TQ], att_t[:])

            # ---------- Phase O: partial out-projection ----------
            with (
                tc.tile_pool(name="ow", bufs=1) as ow,
                tc.tile_pool(name="os", bufs=3) as osb,
                tc.tile_pool(name="ops", bufs=4, space="PSUM") as ops,
            ):
                att_sb = ow.tile([P, KSL, T], F32R)
                nc.sync.dma_start(
                    att_sb[:], attT_d[:].rearrange("s p t -> p s t").bitcast(F32R))
                wout_sb = ow.tile([P, KSL, C], F32R)
                nc.sync.dma_start(wout_sb[:], wout3[:].bitcast(F32R))
                for tb in range(NTB):
                    for co in range(NCO):
                        ps = ops.tile([P, TQ], F32, tag="o")
                        for s in range(KSL):
                            nc.tensor.matmul(
                                ps[:], att_sb[:, s, tb * P:(tb + 1) * P],
                                wout_sb[:, s, co * TQ:(co + 1) * TQ],
                                start=(s == 0), stop=(s == KSL - 1))
                        ev = osb.tile([P, TQ], F32, tag="oev")
                        nc.scalar.copy(ev[:], ps[:])
                        nc.sync.dma_start(
                            yp[tb * P:(tb + 1) * P, co * TQ:(co + 1) * TQ], ev[:])

    nc.compile()
    return nc


_NC_CACHE = {}


def _get_nc(T, C, HPC, HD):
    key = (T, C, HPC, HD)
    if key not in _NC_CACHE:
        _NC_CACHE[key] = build_nc(T, C, HPC, HD)
    return _NC_CACHE[key]


def shard_inputs(x, Wqkv, bqkv, Wout, n_heads, n_cores=8, groups_per_batch=2):
    """Build per-core input maps. Core r -> (b = r//gpb, g = r%gpb)."""
    B, T, C = x.shape
    hd = C // n_heads
    hpc = n_heads // groups_per_batch
    CL = hpc * hd
    Wq, Wk, Wv = Wqkv[:C], Wqkv[C:2 * C], Wqkv[2 * C:]
    bq, bk, bvv = bqkv[:C], bqkv[C:2 * C], bqkv[2 * C:]
    xT = {b: np.ascontiguousarray(x[b].T) for b in range(B)}
    pre_g = {}
    for g in range(groups_per_batch):
        sl = slice(g * CL, (g + 1) * CL)
        wqkT = np.ascontiguousarray(
            np.concatenate([Wq[sl], Wk[sl]], axis=0).T)        # [C, 2CL]
        wvT = np.ascontiguousarray(Wv[sl].T)                   # [C, CL]
        bqk_c = np.ascontiguousarray(
            np.concatenate([bq[sl], bk[sl]]))                  # [2CL]
        bv_c = np.ascontiguousarray(bvv[sl])[None, :]          # [1, CL]
        woutT = np.ascontiguousarray(Wout[:, sl].T)            # [CL, C]
        pre_g[g] = (wqkT, wvT, bqk_c, bv_c, woutT)
    in_maps = []
    for r in range(n_cores):
        b, g = r // groups_per_batch, r % groups_per_batch
        wqkT, wvT, bqk_c, bv_c, woutT = pre_g[g]
        in_maps.append({
            "xT": xT[b], "wqkT": wqkT, "wvT": wvT,
            "bqk": bqk_c, "bv": bv_c, "woutT": woutT,
        })
    return in_maps


def unshard_outputs(results, bout, B=4, n_heads=16, T=2048, C=2048,
                    groups_per_batch=2):
    hd = C // n_heads
    hpc = n_heads // groups_per_batch
    y = np.empty((B, T, C), np.float32)
    new_k = np.empty((B, n_heads, T, hd), np.float32)
    new_v = np.empty((B, n_heads, T, hd), np.float32)
    for b in range(B):
        acc = bout[None, :].astype(np.float32).copy()
        acc = np.broadcast_to(acc, (T, C)).copy()
        for g in range(groups_per_batch):
            r = b * groups_per_batch + g
            acc += results[r]["yp"]
            new_k[b, g * hpc:(g + 1) * hpc] = results[r]["k_out"]
            new_v[b, g * hpc:(g + 1) * hpc] = results[r]["v_out"]
        y[b] = acc
    return y, new_k, new_v


def kernel(x, Wqkv, bqkv, Wout, bout, n_heads):
    x = np.asarray(x, np.float32)
    Wqkv = np.asarray(Wqkv, np.float32)
    bqkv = np.asarray(bqkv, np.float32)
    Wout = np.asarray(Wout, np.float32)
    bout = np.asarray(bout, np.float32)
    n_heads = int(n_heads)

    B, T, C = x.shape
    hd = C // n_heads
    gpb = 2
    hpc = n_heads // gpb
    n_cores = B * gpb

    nc = _get_nc(T, C, hpc, hd)
    in_maps = shard_inputs(x, Wqkv, bqkv, Wout, n_heads, n_cores, gpb)
    res = run_bass_kernel_spmd(nc, in_maps, list(range(n_cores)))
    return unshard_outputs(res.results, bout, B, n_heads, T, C, gpb)
